# revision 7
# baseline (speedup 1.0000x reference)
"""Trainium2 Bass kernel for the gated equivariant tensor-product layer.

Math (per node z, MUL=64):
  x0 = feats[:, :64], x1[u,i] = feats[:, 64+3u+i], a0 = attrs[:,0], a1 = attrs[:,1:4]
  out0 = ALPHA*( (x0*a0) @ W1 + C*(sum_i x1_i*a1_i) @ W2 )          # [N,128] = s|g
  out1_i = ALPHA*( C*(x0*a1_i) @ W3 + C*(x1_i*a0) @ W4 )            # [N,64] per i
  out = [ silu(s) | sigmoid(g)[w]*out1_i[w] at col 64+3w+i ]

Sharding: pure data parallelism over nodes, 8 cores x 25000 nodes
(padded to 25088 = 49 chunks of 512 per core).
"""

import sys
import numpy as np

sys.path.insert(0, "/opt/trn_rl_repo")

MUL = 64
C3 = 1.0 / np.sqrt(3.0)
ALPHA = 1.0 / np.sqrt(MUL * 1 * 2)

N_CORES = 8
N_PER = 25000
N_PAD = 25088          # 49 * 512
CHUNK = 512
N_CHUNKS = N_PAD // CHUNK
G = 4                  # 128-node groups per chunk
P = 128

_BUILT = None


def _build_nc():
    import concourse.bacc as bacc
    import concourse.mybir as mybir
    from concourse.tile import TileContext
    from concourse.masks import make_identity

    f32 = mybir.dt.float32
    MULT = mybir.AluOpType.mult
    ADD = mybir.AluOpType.add
    AF = mybir.ActivationFunctionType

    nc = bacc.Bacc("TRN2", target_bir_lowering=False, debug=False)

    feats_d = nc.declare_dram_parameter("node_feats", [N_PAD, 256], f32, isOutput=False)
    attrs_d = nc.declare_dram_parameter("node_attrs", [N_PAD, 4], f32, isOutput=False)
    w1_d = nc.declare_dram_parameter("W1", [64, 128], f32, isOutput=False)
    w2_d = nc.declare_dram_parameter("W2", [64, 128], f32, isOutput=False)
    w3_d = nc.declare_dram_parameter("W3", [64, 64], f32, isOutput=False)
    w4_d = nc.declare_dram_parameter("W4", [64, 64], f32, isOutput=False)
    out_d = nc.declare_dram_parameter("out", [N_PAD, 256], f32, isOutput=True)

    with TileContext(nc) as tc:
        wpool = tc.alloc_tile_pool(name="wpool", bufs=1)
        io = tc.alloc_tile_pool(name="io", bufs=3)
        stage = tc.alloc_tile_pool(name="stage", bufs=2)
        rhs = tc.alloc_tile_pool(name="rhs", bufs=2)
        post = tc.alloc_tile_pool(name="post", bufs=2)
        ps_fwd = tc.alloc_tile_pool(name="ps_fwd", bufs=2, space="PSUM")
        ps_mm = tc.alloc_tile_pool(name="ps_mm", bufs=1, space="PSUM")
        ps_bwd = tc.alloc_tile_pool(name="ps_bwd", bufs=2, space="PSUM")

        # --- constants / weights (once) ---
        ident = wpool.tile([P, P], f32, tag="ident")
        make_identity(nc, ident)

        # Wc: rows 0:64 = ALPHA*W1, rows 64:128 = ALPHA*C3*W2.  lhsT for out0.
        Wc = wpool.tile([P, 128], f32, tag="Wc")
        nc.sync.dma_start(Wc[0:64, :], w1_d[:, :])
        nc.sync.dma_start(Wc[64:128, :], w2_d[:, :])
        nc.vector.tensor_scalar_mul(Wc[0:64, :], Wc[0:64, :], float(ALPHA))
        nc.vector.tensor_scalar_mul(Wc[64:128, :], Wc[64:128, :], float(ALPHA * C3))

        # LA: rows 0:64 = ALPHA*C3*W3, rows 64:128 = ALPHA*C3*W4 (stacked lhsT:
        # one matmul computes t3_i@W3' + t4_i@W4'' accumulated by the PE).
        LA = wpool.tile([P, 64], f32, tag="LA")
        nc.sync.dma_start(LA[0:64, :], w3_d[:, :])
        nc.sync.dma_start(LA[64:128, :], w4_d[:, :])
        nc.vector.tensor_scalar_mul(LA[0:64, :], LA[0:64, :], float(ALPHA * C3))
        nc.vector.tensor_scalar_mul(LA[64:128, :], LA[64:128, :], float(ALPHA * C3))

        # --- per-chunk pipeline ---
        for ch in range(N_CHUNKS):
            z0 = ch * CHUNK
            F = io.tile([P, G, 256], f32, tag="feats")
            A = io.tile([P, G, 4], f32, tag="attrs")
            nc.sync.dma_start(
                F[:], feats_d[z0 : z0 + CHUNK, :].rearrange("(g p) c -> p g c", p=P)
            )
            nc.sync.dma_start(
                A[:], attrs_d[z0 : z0 + CHUNK, :].rearrange("(g p) c -> p g c", p=P)
            )

            # staging tile S: 8 k-blocks of 64 cols:
            #   [t0 | d | t3_0 | t4_0 | t3_1 | t4_1 | t3_2 | t4_2]
            S = stage.tile([P, G, 512], f32, tag="S")
            DT = stage.tile([P, G, 192], f32, tag="DT")

            # t0 = x0 * a0
            nc.vector.tensor_tensor(
                S[:, :, 0:64],
                F[:, :, 0:64],
                A[:, :, 0:1].to_broadcast([P, G, 64]),
                MULT,
            )
            # d = sum_i x1_i * a1_i  (product then 3-way reduce)
            nc.vector.tensor_tensor(
                DT[:].rearrange("p g (u i) -> p g u i", i=3),
                F[:, :, 64:256].rearrange("p g (u i) -> p g u i", i=3),
                A[:, :, None, 1:4].to_broadcast([P, G, 64, 3]),
                MULT,
            )
            nc.vector.tensor_reduce(
                S[:, :, 64:128],
                DT[:].rearrange("p g (u i) -> p g u i", i=3),
                axis=mybir.AxisListType.X,
                op=ADD,
            )
            # t3_i = x0 * a1_i   -> S cols 128+128i .. 128+128i+64
            t3_view = S[:, :, 128:512].rearrange("p g (i h u) -> p g i h u", i=3, h=2)
            nc.gpsimd.tensor_tensor(
                t3_view[:, :, :, 0, :],
                F[:, :, None, 0:64].to_broadcast([P, G, 3, 64]),
                A[:, :, 1:4, None].to_broadcast([P, G, 3, 64]),
                MULT,
            )
            # t4_i = x1_i * a0   -> S cols 192+128i .. 192+128i+64
            t4_view = S[:, :, 128:512].rearrange("p g (i h u) -> p g i h u", i=3, h=2)
            nc.gpsimd.tensor_tensor(
                t4_view[:, :, :, 1, :].rearrange("p g i u -> p g u i"),
                F[:, :, 64:256].rearrange("p g (u i) -> p g u i", i=3),
                A[:, :, 0:1, None].to_broadcast([P, G, 64, 3]),
                MULT,
            )

            # forward transposes: 4 k-block-pairs x 4 groups -> psum, then copy to SBUF rhs
            Rt = []
            for b in range(4):
                FT = ps_fwd.tile([P, CHUNK], f32, tag="ft")
                for g in range(G):
                    nc.tensor.transpose(
                        FT[:, g * P : (g + 1) * P],
                        S[:, g, b * 128 : (b + 1) * 128],
                        ident,
                    )
                R = rhs.tile([P, CHUNK], f32, tag=f"r{b}")
                if b % 2 == 0:
                    nc.scalar.copy(R[:], FT[:])
                else:
                    nc.vector.tensor_copy(R[:], FT[:])
                Rt.append(R)

            # matmuls
            O1 = ps_mm.tile([P, CHUNK], f32, tag="O1")   # [s ; g]
            O2 = ps_mm.tile([P, CHUNK], f32, tag="O2")   # [g ; -]
            P1 = ps_mm.tile([P, CHUNK], f32, tag="P1")   # [out1_1 ; out1_0]
            P2 = ps_mm.tile([P, CHUNK], f32, tag="P2")   # [ -     ; out1_2]
            nc.tensor.matmul(O1[:, :], Wc[:, :], Rt[0][:, :])
            nc.tensor.matmul(O2[0:64, :], Wc[:, 64:128], Rt[0][:, :])
            nc.tensor.matmul(P1[64:128, :], LA[:, :], Rt[1][:, :])
            nc.tensor.matmul(P1[0:64, :], LA[:, :], Rt[2][:, :])
            nc.tensor.matmul(P2[64:128, :], LA[:, :], Rt[3][:, :])

            # activations + gating (feature-major)
            U = post.tile([P, CHUNK], f32, tag="U")      # sigmoid(g) on both halves
            B1 = post.tile([P, CHUNK], f32, tag="B1")    # [silu(s) ; gated_0]
            B2 = post.tile([P, CHUNK], f32, tag="B2")    # [gated_1 ; gated_2]
            nc.scalar.activation(U[0:64, :], O2[0:64, :], AF.Sigmoid)
            nc.scalar.activation(U[64:128, :], O1[64:128, :], AF.Sigmoid)
            nc.scalar.activation(B1[0:64, :], O1[0:64, :], AF.Silu)
            nc.vector.tensor_tensor(B1[64:128, :], P1[64:128, :], U[64:128, :], MULT)
            nc.vector.tensor_tensor(B2[0:64, :], P1[0:64, :], U[0:64, :], MULT)
            nc.vector.tensor_tensor(B2[64:128, :], P2[64:128, :], U[64:128, :], MULT)

            # backward transposes -> node-major psum
            BT1 = ps_bwd.tile([P, CHUNK], f32, tag="bt")
            BT2 = ps_bwd.tile([P, CHUNK], f32, tag="bt")
            for g in range(G):
                nc.tensor.transpose(
                    BT1[:, g * P : (g + 1) * P], B1[:, g * P : (g + 1) * P], ident
                )
            for g in range(G):
                nc.tensor.transpose(
                    BT2[:, g * P : (g + 1) * P], B2[:, g * P : (g + 1) * P], ident
                )

            # assemble node-major output, interleaving gated cols 64+3w+i
            OB = io.tile([P, G, 256], f32, tag="outbuf")
            b1v = BT1[:].rearrange("p (g c) -> p g c", g=G)
            b2v = BT2[:].rearrange("p (g c) -> p g c", g=G)
            gv = OB[:, :, 64:256].rearrange("p g (w i) -> p g i w", i=3)
            nc.scalar.copy(OB[:, :, 0:64], b1v[:, :, 0:64])
            nc.vector.tensor_copy(gv[:, :, 0, :], b1v[:, :, 64:128])
            nc.scalar.copy(gv[:, :, 1, :], b2v[:, :, 0:64])
            nc.vector.tensor_copy(gv[:, :, 2, :], b2v[:, :, 64:128])

            nc.sync.dma_start(
                out_d[z0 : z0 + CHUNK, :].rearrange("(g p) c -> p g c", p=P),
                OB[:],
            )

        for pool in (ps_bwd, ps_mm, ps_fwd, post, rhs, stage, io, wpool):
            pool.release()

    nc.compile()
    return nc


def _get_nc():
    global _BUILT
    if _BUILT is None:
        _BUILT = _build_nc()
    return _BUILT


def kernel(node_feats, node_attrs, W1, W2, W3, W4):
    from concourse.bass_utils import run_bass_kernel_spmd

    nc = _get_nc()

    node_feats = np.ascontiguousarray(node_feats, dtype=np.float32)
    node_attrs = np.ascontiguousarray(node_attrs, dtype=np.float32)
    in_maps = []
    for c in range(N_CORES):
        f = node_feats[c * N_PER : (c + 1) * N_PER]
        a = node_attrs[c * N_PER : (c + 1) * N_PER]
        fpad = np.zeros((N_PAD, 256), np.float32)
        apad = np.zeros((N_PAD, 4), np.float32)
        fpad[:N_PER] = f
        apad[:N_PER] = a
        in_maps.append(
            {
                "node_feats": fpad,
                "node_attrs": apad,
                "W1": np.ascontiguousarray(W1, np.float32),
                "W2": np.ascontiguousarray(W2, np.float32),
                "W3": np.ascontiguousarray(W3, np.float32),
                "W4": np.ascontiguousarray(W4, np.float32),
            }
        )

    res = run_bass_kernel_spmd(nc, in_maps, list(range(N_CORES)))
    outs = [r["out"][:N_PER] for r in res.results]
    return np.concatenate(outs, axis=0)


# revision 9
# speedup vs baseline: 1.2808x; 1.2808x over previous
"""Trainium2 Bass kernel for the gated equivariant tensor-product layer.

Math (per node z, MUL=64):
  x0 = feats[:, :64], x1[u,i] = feats[:, 64+3u+i], a0 = attrs[:,0], a1 = attrs[:,1:4]
  out0 = ALPHA*( (x0*a0) @ W1 + C*(sum_i x1_i*a1_i) @ W2 )          # [N,128] = s|g
  out1_i = ALPHA*C*( (x0*a1_i) @ W3 + (x1_i*a0) @ W4 )              # [N,64] per i
  out = [ silu(s) | sigmoid(g)[w]*out1_i[w] at col 64+3w+i ]

Design notes:
 - node-major staging (per-node scalars broadcast along free dims), fp16
 - the i-sum of the W2 path is folded into the matmul contraction by
   replicating W2 rows (dt blocks), so no on-chip reduction is needed
 - stacked lhsT [W3; W4] sums both tensor-product paths in PSUM
 - PE transposes to/from feature-major; matmuls in fp16 (fp32 accum)
 - ACT runs Sigmoid only (no act-table thrash); silu = s*sigmoid(s) on DVE

Sharding: pure data parallelism over nodes, 8 cores x 25000 nodes
(padded to 25088 = 49 chunks of 512 per core).
"""

import sys
import numpy as np

sys.path.insert(0, "/opt/trn_rl_repo")

MUL = 64
C3 = 1.0 / np.sqrt(3.0)
ALPHA = 1.0 / np.sqrt(MUL * 1 * 2)

N_CORES = 8
N_PER = 25000
N_PAD = 25088          # 49 * 512
CHUNK = 512
N_CHUNKS = N_PAD // CHUNK
G = 4                  # 128-node groups per chunk
P = 128

_BUILT = None


def _build_nc():
    import concourse.bacc as bacc
    import concourse.mybir as mybir
    from concourse.tile import TileContext
    from concourse.masks import make_identity

    f32 = mybir.dt.float32
    f16 = mybir.dt.float16
    MULT = mybir.AluOpType.mult
    AF = mybir.ActivationFunctionType

    nc = bacc.Bacc("TRN2", target_bir_lowering=False, debug=False)

    feats_d = nc.declare_dram_parameter("node_feats", [N_PAD, 256], f32, isOutput=False)
    attrs_d = nc.declare_dram_parameter("node_attrs", [N_PAD, 4], f32, isOutput=False)
    w1_d = nc.declare_dram_parameter("W1", [64, 128], f32, isOutput=False)
    w2_d = nc.declare_dram_parameter("W2", [64, 128], f32, isOutput=False)
    w3_d = nc.declare_dram_parameter("W3", [64, 64], f32, isOutput=False)
    w4_d = nc.declare_dram_parameter("W4", [64, 64], f32, isOutput=False)
    out_d = nc.declare_dram_parameter("out", [N_PAD, 256], f32, isOutput=True)

    with TileContext(nc) as tc:
        wpool = tc.alloc_tile_pool(name="wpool", bufs=1)
        io = tc.alloc_tile_pool(name="io", bufs=3)
        stage = tc.alloc_tile_pool(name="stage", bufs=2)
        rhs = tc.alloc_tile_pool(name="rhs", bufs=2)
        post = tc.alloc_tile_pool(name="post", bufs=2)
        ps_fwd = tc.alloc_tile_pool(name="ps_fwd", bufs=2, space="PSUM")
        ps_mm = tc.alloc_tile_pool(name="ps_mm", bufs=1, space="PSUM")
        ps_bwd = tc.alloc_tile_pool(name="ps_bwd", bufs=2, space="PSUM")

        # --- constants / weights (once) ---
        ident16 = wpool.tile([P, P], f16, tag="ident16")
        make_identity(nc, ident16)
        ident32 = wpool.tile([P, P], f32, tag="ident32")
        make_identity(nc, ident32)

        # fp32 staging for scaling, then cast to fp16 lhsT tiles.
        wtmp = wpool.tile([P, 128], f32, tag="wtmp")
        nc.sync.dma_start(wtmp[0:64, :], w1_d[:, :])
        nc.sync.dma_start(wtmp[64:128, :], w2_d[:, :])
        nc.vector.tensor_scalar_mul(wtmp[0:64, :], wtmp[0:64, :], float(ALPHA))
        nc.vector.tensor_scalar_mul(wtmp[64:128, :], wtmp[64:128, :], float(ALPHA * C3))

        # Wc0 = [alpha*W1 ; alpha*C3*W2], Wc4 = [alpha*C3*W2 ; alpha*C3*W2]
        Wc0 = wpool.tile([P, 128], f16, tag="Wc0")
        Wc4 = wpool.tile([P, 128], f16, tag="Wc4")
        nc.vector.tensor_copy(Wc0[:, :], wtmp[:, :])
        nc.scalar.copy(Wc4[0:64, :], wtmp[64:128, :])
        nc.scalar.copy(Wc4[64:128, :], wtmp[64:128, :])

        wtmp2 = wpool.tile([P, 64], f32, tag="wtmp2")
        nc.sync.dma_start(wtmp2[0:64, :], w3_d[:, :])
        nc.sync.dma_start(wtmp2[64:128, :], w4_d[:, :])
        nc.vector.tensor_scalar_mul(wtmp2[0:64, :], wtmp2[0:64, :], float(ALPHA * C3))
        nc.vector.tensor_scalar_mul(
            wtmp2[64:128, :], wtmp2[64:128, :], float(ALPHA * C3)
        )
        LA = wpool.tile([P, 64], f16, tag="LA")
        nc.vector.tensor_copy(LA[:, :], wtmp2[:, :])

        # attrs for the whole core, loaded once: [p, chunk, g, 4]
        AA = wpool.tile([P, N_CHUNKS, G, 4], f32, tag="AA")
        nc.sync.dma_start(
            AA[:], attrs_d[:, :].rearrange("(c g p) a -> p c g a", p=P, g=G)
        )

        # --- per-chunk pipeline ---
        for ch in range(N_CHUNKS):
            z0 = ch * CHUNK
            F = io.tile([P, G, 256], f32, tag="feats")
            nc.sync.dma_start(
                F[:], feats_d[z0 : z0 + CHUNK, :].rearrange("(g p) c -> p g c", p=P)
            )
            A = AA[:, ch]  # [128, G, 4]

            # staging tile S (fp16), per-g column layout (640 cols):
            #   [ t0 | dt_0 dt_1 dt_2 | t3_0 t4_0 | t3_1 t4_1 | t3_2 t4_2 ]
            S = stage.tile([P, G, 640], f16, tag="S")

            # t0 = x0 * a0                                   (DVE)
            nc.vector.tensor_tensor(
                S[:, :, 0:64],
                F[:, :, 0:64],
                A[:, :, 0:1].to_broadcast([P, G, 64]),
                MULT,
            )
            # dt_i[u] = x1[u,i] * a1_i  (de-interleaved)     (GPSIMD)
            nc.gpsimd.tensor_tensor(
                S[:, :, 64:256].rearrange("p g (i u) -> p g u i", i=3),
                F[:, :, 64:256].rearrange("p g (u i) -> p g u i", i=3),
                A[:, :, None, 1:4].to_broadcast([P, G, 64, 3]),
                MULT,
            )
            t34 = S[:, :, 256:640].rearrange("p g (i h u) -> p g i h u", i=3, h=2)
            # t3_i = x0 * a1_i                               (DVE)
            nc.vector.tensor_tensor(
                t34[:, :, :, 0, :],
                F[:, :, None, 0:64].to_broadcast([P, G, 3, 64]),
                A[:, :, 1:4, None].to_broadcast([P, G, 3, 64]),
                MULT,
            )
            # t4_i = x1_i * a0                               (GPSIMD)
            nc.gpsimd.tensor_tensor(
                t34[:, :, :, 1, :],
                F[:, :, 64:256].rearrange("p g (u i) -> p g i u", i=3),
                A[:, :, 0:1, None].to_broadcast([P, G, 3, 64]),
                MULT,
            )

            # forward transposes (PE, fp16): 5 blocks x 4 groups
            Rt = rhs.tile([P, 5, CHUNK], f16, tag="R")
            for b in range(5):
                FT = ps_fwd.tile([P, CHUNK], f16, tag="ft")
                for g in range(G):
                    nc.tensor.transpose(
                        FT[:, g * P : (g + 1) * P],
                        S[:, g, b * 128 : (b + 1) * 128],
                        ident16,
                    )
                if b in (0, 2):
                    nc.vector.tensor_copy(Rt[:, b, :], FT[:])
                else:
                    nc.scalar.copy(Rt[:, b, :], FT[:])

            # matmuls (fp16 in, fp32 accum)
            # blocks: R0=[t0|dt_0] R1=[dt_1|dt_2] R2=[t3_0|t4_0] R3=[t3_1|t4_1] R4=[t3_2|t4_2]
            O1 = ps_mm.tile([P, CHUNK], f32, tag="O1")   # [s ; g]
            P1 = ps_mm.tile([P, CHUNK], f32, tag="P1")   # [out1_1 ; out1_0]
            P2 = ps_mm.tile([P, CHUNK], f32, tag="P2")   # [g-dup ; out1_2]
            nc.tensor.matmul(O1[:, :], Wc0[:, :], Rt[:, 0, :], start=True, stop=False)
            nc.tensor.matmul(O1[:, :], Wc4[:, :], Rt[:, 1, :], start=False, stop=True)
            nc.tensor.matmul(
                P2[0:64, :], Wc0[:, 64:128], Rt[:, 0, :], start=True, stop=False
            )
            nc.tensor.matmul(
                P2[0:64, :], Wc4[:, 64:128], Rt[:, 1, :], start=False, stop=True
            )
            nc.tensor.matmul(P1[0:64, :], LA[:, :], Rt[:, 3, :])
            nc.tensor.matmul(P1[64:128, :], LA[:, :], Rt[:, 2, :])
            nc.tensor.matmul(P2[64:128, :], LA[:, :], Rt[:, 4, :])

            # sigmoids (ACT only ever runs Sigmoid -> no act-table reloads)
            U = post.tile([P, CHUNK], f32, tag="U")      # sigmoid(g), both halves
            SGS = post.tile([64, CHUNK], f32, tag="SGS")  # sigmoid(s)
            nc.scalar.activation(U[0:64, :], P2[0:64, :], AF.Sigmoid)
            nc.scalar.activation(U[64:128, :], O1[64:128, :], AF.Sigmoid)
            nc.scalar.activation(SGS[:, :], O1[0:64, :], AF.Sigmoid)

            # gating (DVE, psum x sbuf)
            BA = post.tile([P, CHUNK], f32, tag="BA")    # [gated_1 ; gated_0]
            BB = post.tile([P, CHUNK], f32, tag="BB")    # [silu ; gated_2]
            nc.vector.tensor_tensor(BA[:, :], P1[:, :], U[:, :], MULT)
            nc.vector.tensor_tensor(BB[0:64, :], O1[0:64, :], SGS[:, :], MULT)
            nc.vector.tensor_tensor(BB[64:128, :], P2[64:128, :], U[64:128, :], MULT)

            # backward transposes (PE, fp32) -> node-major psum
            BTA = ps_bwd.tile([P, CHUNK], f32, tag="bt")
            BTB = ps_bwd.tile([P, CHUNK], f32, tag="bt")
            for g in range(G):
                nc.tensor.transpose(
                    BTA[:, g * P : (g + 1) * P], BA[:, g * P : (g + 1) * P], ident32
                )
            for g in range(G):
                nc.tensor.transpose(
                    BTB[:, g * P : (g + 1) * P], BB[:, g * P : (g + 1) * P], ident32
                )

            # final node-major assembly (interleave gated cols 64+3w+i)
            OB = io.tile([P, G, 256], f32, tag="outbuf")
            bav = BTA[:].rearrange("p (g c) -> p g c", g=G)
            bbv = BTB[:].rearrange("p (g c) -> p g c", g=G)
            # BTA cols: [gated_1 | gated_0] -> out cols 64+3w+1 / 64+3w+0
            gpair = OB[:, :, 64:256].rearrange("p g (w i) -> p g i w", i=3)
            nc.vector.tensor_copy(gpair[:, :, 1, :], bav[:, :, 0:64])
            nc.vector.tensor_copy(gpair[:, :, 0, :], bav[:, :, 64:128])
            nc.scalar.copy(OB[:, :, 0:64], bbv[:, :, 0:64])
            nc.scalar.copy(gpair[:, :, 2, :], bbv[:, :, 64:128])

            nc.sync.dma_start(
                out_d[z0 : z0 + CHUNK, :].rearrange("(g p) c -> p g c", p=P),
                OB[:],
            )

        for pool in (ps_bwd, ps_mm, ps_fwd, post, rhs, stage, io, wpool):
            pool.release()

    nc.compile()
    return nc


def _get_nc():
    global _BUILT
    if _BUILT is None:
        _BUILT = _build_nc()
    return _BUILT


def kernel(node_feats, node_attrs, W1, W2, W3, W4):
    from concourse.bass_utils import run_bass_kernel_spmd

    nc = _get_nc()

    node_feats = np.ascontiguousarray(node_feats, dtype=np.float32)
    node_attrs = np.ascontiguousarray(node_attrs, dtype=np.float32)
    in_maps = []
    for c in range(N_CORES):
        f = node_feats[c * N_PER : (c + 1) * N_PER]
        a = node_attrs[c * N_PER : (c + 1) * N_PER]
        fpad = np.zeros((N_PAD, 256), np.float32)
        apad = np.zeros((N_PAD, 4), np.float32)
        fpad[:N_PER] = f
        apad[:N_PER] = a
        in_maps.append(
            {
                "node_feats": fpad,
                "node_attrs": apad,
                "W1": np.ascontiguousarray(W1, np.float32),
                "W2": np.ascontiguousarray(W2, np.float32),
                "W3": np.ascontiguousarray(W3, np.float32),
                "W4": np.ascontiguousarray(W4, np.float32),
            }
        )

    res = run_bass_kernel_spmd(nc, in_maps, list(range(N_CORES)))
    outs = [r["out"][:N_PER] for r in res.results]
    return np.concatenate(outs, axis=0)
